# revision 15
# baseline (speedup 1.0000x reference)
"""GroupProjection Trainium2 kernel.

y[b,t,g,:] = x[b,t,idx[g]] @ W[g] + bias[g], output [B,T,G*GO].

Strategy:
  - Fold the per-group gather+block-diagonal matmul into one dense matmul:
    Wbig[F, G*GO], Wbig[idx[g,f], g*GO+o] += W[g,f,o].  y = x @ Wbig + b.
  - Data-parallel over the batch axis: 8 cores x 32 stocks, 16384 tokens/core.
  - Per core: tile tokens by 128.  PE transposes x tiles ([tok,f] -> [f,tok]),
    then two K=128 float32r matmuls accumulate y[tok, 512] in PSUM.
    ScalarE evicts the transposed tiles PSUM->SBUF; VectorE fuses the bias add
    into the y PSUM->SBUF eviction.  DMAs batched per 1024-token megatile.

Hardcoded shapes: x [256, 512, 256] f32, W [8, 32, 64], b [8, 64], idx [8, 32].
"""

import numpy as np

B, T, F = 256, 512, 256
G, GF, GO = 8, 32, 64
NOUT = G * GO  # 512
N_CORES = 8
NTOK = (B // N_CORES) * T  # 16384 tokens per core
SUB = 128                  # tokens per subtile (partition dim)
LOAD_SUBS = 8              # subtiles per input DMA (1MB, 8KB/partition)
STORE_SUBS = 4             # subtiles per output DMA (1MB, 8KB/partition)
MEGA = SUB * LOAD_SUBS     # 1024 tokens per load block
N_MEGA = NTOK // MEGA      # 16
# Token mapping within a load block: token = tok0 + p*LOAD_SUBS + c
# (partition-major), so each partition's load/store is one contiguous
# HBM chunk (8KB in / 8KB out per partition per DMA).

_CACHE = {}


def _build_module():
    import concourse.mybir as mybir
    import concourse.tile as tile
    from concourse import bacc

    f32 = mybir.dt.float32
    f32r = mybir.dt.float32r

    nc = bacc.Bacc("TRN2", target_bir_lowering=False, debug=False)
    x_d = nc.declare_dram_parameter("x", [NTOK, F], f32, isOutput=False)
    w_d = nc.declare_dram_parameter("w", [128, 2 * NOUT], f32r, isOutput=False)
    b_d = nc.declare_dram_parameter("b", [128, NOUT], f32, isOutput=False)
    id_d = nc.declare_dram_parameter("ident", [128, 128], f32, isOutput=False)
    y_d = nc.declare_dram_parameter("y", [NTOK, NOUT], f32, isOutput=True)

    with tile.TileContext(nc) as tc:
        with (
            tc.tile_pool(name="const", bufs=1) as const_pool,
            tc.tile_pool(name="xin", bufs=3) as xin_pool,
            tc.tile_pool(name="xt", bufs=4) as xt_pool,
            tc.tile_pool(name="yout", bufs=4) as y_pool,
            tc.tile_pool(name="tp", bufs=4, space="PSUM") as tp_pool,
            tc.tile_pool(name="yp", bufs=2, space="PSUM") as yp_pool,
        ):
            w_sb = const_pool.tile([128, 2 * NOUT], f32r)
            nc.sync.dma_start(out=w_sb[:], in_=w_d[:])
            b_sb = const_pool.tile([128, NOUT], f32)
            nc.sync.dma_start(out=b_sb[:], in_=b_d[:])
            id_sb = const_pool.tile([128, 128], f32)
            nc.sync.dma_start(out=id_sb[:], in_=id_d[:])

            for mt in range(N_MEGA):
                tok0 = mt * MEGA
                x_in = xin_pool.tile([128, LOAD_SUBS * F], f32)
                nc.sync.dma_start(
                    out=x_in.rearrange("p (c f) -> p c f", c=LOAD_SUBS),
                    in_=x_d[tok0 : tok0 + MEGA, :].rearrange(
                        "(p c) f -> p c f", p=128
                    ),
                )
                y_sb = None
                for s in range(LOAD_SUBS):
                    if s % STORE_SUBS == 0:
                        y_sb = y_pool.tile([128, STORE_SUBS * NOUT], f32)
                    so = s % STORE_SUBS
                    xt = xt_pool.tile([128, F], f32r)
                    for h in range(2):
                        tp = tp_pool.tile([128, 128], f32)
                        nc.tensor.transpose(
                            tp[:],
                            x_in[:, s * F + h * 128 : s * F + (h + 1) * 128],
                            id_sb[:],
                        )
                        nc.scalar.copy(
                            out=xt[:, h * 128 : (h + 1) * 128], in_=tp[:]
                        )
                    yp = yp_pool.tile([128, NOUT], f32)
                    nc.tensor.matmul(
                        yp[:],
                        lhsT=xt[:, 0:128],
                        rhs=w_sb[:, 0:NOUT],
                        start=True,
                        stop=False,
                    )
                    nc.tensor.matmul(
                        yp[:],
                        lhsT=xt[:, 128:256],
                        rhs=w_sb[:, NOUT : 2 * NOUT],
                        start=False,
                        stop=True,
                    )
                    nc.vector.tensor_add(
                        out=y_sb[:, so * NOUT : (so + 1) * NOUT],
                        in0=yp[:],
                        in1=b_sb[:],
                    )
                    if so == STORE_SUBS - 1:
                        g0 = s - (STORE_SUBS - 1)
                        nc.sync.dma_start(
                            out=y_d[tok0 : tok0 + MEGA, :].rearrange(
                                "(p c) o -> p c o", p=128
                            )[:, g0 : g0 + STORE_SUBS, :],
                            in_=y_sb.rearrange(
                                "p (c o) -> p c o", c=STORE_SUBS
                            ),
                        )
    nc.finalize()
    return nc


def _get_nc():
    if "nc" not in _CACHE:
        _CACHE["nc"] = _build_module()
    return _CACHE["nc"]


def _prep_inputs(x, W, b, idx):
    x = np.ascontiguousarray(np.asarray(x, dtype=np.float32))
    W = np.asarray(W, dtype=np.float32)
    b = np.asarray(b, dtype=np.float32)
    idx = np.asarray(idx)

    wbig = np.zeros((F, NOUT), dtype=np.float32)
    for g in range(G):
        np.add.at(wbig[:, g * GO : (g + 1) * GO], idx[g].astype(np.int64), W[g])
    w_packed = np.ascontiguousarray(
        np.concatenate([wbig[:128, :], wbig[128:, :]], axis=1)
    )
    b_rep = np.ascontiguousarray(
        np.broadcast_to(b.reshape(1, NOUT), (128, NOUT)).astype(np.float32)
    )
    ident = np.eye(128, dtype=np.float32)

    xs = x.reshape(B * T, F)
    in_maps = []
    for i in range(N_CORES):
        in_maps.append(
            {
                "x": xs[i * NTOK : (i + 1) * NTOK],
                "w": w_packed,
                "b": b_rep,
                "ident": ident,
            }
        )
    return in_maps


def run(inputs, trace=False, **trace_kwargs):
    """Run the SPMD kernel on 8 cores. Returns (full_output, BassKernelResults)."""
    from concourse.bass_utils import run_bass_kernel_spmd

    in_maps = _prep_inputs(
        inputs["x"], inputs["W"], inputs["b"], inputs["idx"]
    )
    nc = _get_nc()
    res = run_bass_kernel_spmd(
        nc, in_maps, list(range(N_CORES)), trace=trace, **trace_kwargs
    )
    out = np.empty((B, T, NOUT), dtype=np.float32)
    bs = B // N_CORES
    for i in range(N_CORES):
        out[i * bs : (i + 1) * bs] = res.results[i]["y"].reshape(bs, T, NOUT)
    return out, res


def kernel(**inputs):
    out, _ = run(inputs, trace=False)
    return out


# revision 17
# speedup vs baseline: 1.2460x; 1.2460x over previous
"""GroupProjection Trainium2 kernel.

y[b,t,g,:] = x[b,t,idx[g]] @ W[g] + bias[g], output [B,T,G*GO].

Strategy:
  - Fold the per-group gather+block-diagonal matmul into one dense matmul:
    Wbig[F, G*GO], Wbig[idx[g,f], g*GO+o] += W[g,f,o].  y = x @ Wbig + b.
  - Data-parallel over the batch axis: 8 cores x 32 stocks, 16384 tokens/core.
  - Per core: tile tokens by 128.  PE transposes x tiles ([tok,f] -> [f,tok]),
    then two K=128 float32r matmuls accumulate y[tok, 512] in PSUM.
    ScalarE evicts the transposed tiles PSUM->SBUF; VectorE fuses the bias add
    into the y PSUM->SBUF eviction.  DMAs batched per 1024-token megatile.

Hardcoded shapes: x [256, 512, 256] f32, W [8, 32, 64], b [8, 64], idx [8, 32].
"""

import numpy as np

B, T, F = 256, 512, 256
G, GF, GO = 8, 32, 64
NOUT = G * GO  # 512
N_CORES = 8
NTOK = (B // N_CORES) * T  # 16384 tokens per core
SUB = 128                  # tokens per subtile (partition dim)
LOAD_SUBS = 8              # subtiles per input DMA (1MB, 8KB/partition)
STORE_SUBS = 4             # subtiles per output DMA (1MB, 8KB/partition)
MEGA = SUB * LOAD_SUBS     # 1024 tokens per load block
N_MEGA = NTOK // MEGA      # 16
# Token mapping within a load block: token = tok0 + p*LOAD_SUBS + c
# (partition-major), so each partition's load/store is one contiguous
# HBM chunk (8KB in / 8KB out per partition per DMA).

_CACHE = {}


def _build_module():
    import concourse.mybir as mybir
    import concourse.tile as tile
    from concourse import bacc

    f32 = mybir.dt.float32
    f32r = mybir.dt.float32r

    nc = bacc.Bacc("TRN2", target_bir_lowering=False, debug=False)
    x_d = nc.declare_dram_parameter("x", [NTOK, F], f32, isOutput=False)
    w_d = nc.declare_dram_parameter("w", [128, 2 * NOUT], f32r, isOutput=False)
    b_d = nc.declare_dram_parameter("b", [128, NOUT], f32, isOutput=False)
    id_d = nc.declare_dram_parameter("ident", [128, 128], f32, isOutput=False)
    y_d = nc.declare_dram_parameter("y", [NTOK, NOUT], f32, isOutput=True)

    with tile.TileContext(nc) as tc:
        with (
            tc.tile_pool(name="const", bufs=1) as const_pool,
            tc.tile_pool(name="xin", bufs=4) as xin_pool,
            tc.tile_pool(name="xt", bufs=4) as xt_pool,
            tc.tile_pool(name="yout", bufs=4) as y_pool,
            tc.tile_pool(name="tp", bufs=4, space="PSUM") as tp_pool,
            tc.tile_pool(name="yp", bufs=2, space="PSUM") as yp_pool,
        ):
            w_sb = const_pool.tile([128, 2 * NOUT], f32r)
            nc.sync.dma_start(out=w_sb[:], in_=w_d[:])
            b_sb = const_pool.tile([128, NOUT], f32)
            nc.sync.dma_start(out=b_sb[:], in_=b_d[:])
            id_sb = const_pool.tile([128, 128], f32)
            nc.sync.dma_start(out=id_sb[:], in_=id_d[:])

            for mt in range(N_MEGA):
                tok0 = mt * MEGA
                x_in = xin_pool.tile([128, LOAD_SUBS * F], f32)
                nc.sync.dma_start(
                    out=x_in.rearrange("p (c f) -> p c f", c=LOAD_SUBS),
                    in_=x_d[tok0 : tok0 + MEGA, :].rearrange(
                        "(p c) f -> p c f", p=128
                    ),
                )
                y_sb = None
                for s in range(LOAD_SUBS):
                    if s % STORE_SUBS == 0:
                        y_sb = y_pool.tile([128, STORE_SUBS * NOUT], f32)
                    so = s % STORE_SUBS
                    xt = xt_pool.tile([128, F], f32r)
                    for h in range(2):
                        tp = tp_pool.tile([128, 128], f32)
                        nc.tensor.transpose(
                            tp[:],
                            x_in[:, s * F + h * 128 : s * F + (h + 1) * 128],
                            id_sb[:],
                        )
                        nc.scalar.copy(
                            out=xt[:, h * 128 : (h + 1) * 128], in_=tp[:]
                        )
                    yp = yp_pool.tile([128, NOUT], f32)
                    nc.tensor.matmul(
                        yp[:],
                        lhsT=xt[:, 0:128],
                        rhs=w_sb[:, 0:NOUT],
                        start=True,
                        stop=False,
                    )
                    nc.tensor.matmul(
                        yp[:],
                        lhsT=xt[:, 128:256],
                        rhs=w_sb[:, NOUT : 2 * NOUT],
                        start=False,
                        stop=True,
                    )
                    nc.vector.tensor_add(
                        out=y_sb[:, so * NOUT : (so + 1) * NOUT],
                        in0=yp[:],
                        in1=b_sb[:],
                    )
                    if so == STORE_SUBS - 1:
                        g0 = s - (STORE_SUBS - 1)
                        nc.scalar.dma_start(
                            out=y_d[tok0 : tok0 + MEGA, :].rearrange(
                                "(p c) o -> p c o", p=128
                            )[:, g0 : g0 + STORE_SUBS, :],
                            in_=y_sb.rearrange(
                                "p (c o) -> p c o", c=STORE_SUBS
                            ),
                        )
    nc.finalize()
    return nc


def _get_nc():
    if "nc" not in _CACHE:
        _CACHE["nc"] = _build_module()
    return _CACHE["nc"]


def _prep_inputs(x, W, b, idx):
    x = np.ascontiguousarray(np.asarray(x, dtype=np.float32))
    W = np.asarray(W, dtype=np.float32)
    b = np.asarray(b, dtype=np.float32)
    idx = np.asarray(idx)

    wbig = np.zeros((F, NOUT), dtype=np.float32)
    for g in range(G):
        np.add.at(wbig[:, g * GO : (g + 1) * GO], idx[g].astype(np.int64), W[g])
    w_packed = np.ascontiguousarray(
        np.concatenate([wbig[:128, :], wbig[128:, :]], axis=1)
    )
    b_rep = np.ascontiguousarray(
        np.broadcast_to(b.reshape(1, NOUT), (128, NOUT)).astype(np.float32)
    )
    ident = np.eye(128, dtype=np.float32)

    xs = x.reshape(B * T, F)
    in_maps = []
    for i in range(N_CORES):
        in_maps.append(
            {
                "x": xs[i * NTOK : (i + 1) * NTOK],
                "w": w_packed,
                "b": b_rep,
                "ident": ident,
            }
        )
    return in_maps


def run(inputs, trace=False, **trace_kwargs):
    """Run the SPMD kernel on 8 cores. Returns (full_output, BassKernelResults)."""
    from concourse.bass_utils import run_bass_kernel_spmd

    in_maps = _prep_inputs(
        inputs["x"], inputs["W"], inputs["b"], inputs["idx"]
    )
    nc = _get_nc()
    res = run_bass_kernel_spmd(
        nc, in_maps, list(range(N_CORES)), trace=trace, **trace_kwargs
    )
    out = np.empty((B, T, NOUT), dtype=np.float32)
    bs = B // N_CORES
    for i in range(N_CORES):
        out[i * bs : (i + 1) * bs] = res.results[i]["y"].reshape(bs, T, NOUT)
    return out, res


def kernel(**inputs):
    out, _ = run(inputs, trace=False)
    return out


# revision 20
# speedup vs baseline: 1.2852x; 1.0314x over previous
"""GroupProjection Trainium2 kernel.

y[b,t,g,:] = x[b,t,idx[g]] @ W[g] + bias[g], output [B,T,G*GO].

Strategy:
  - Fold the per-group gather+block-diagonal matmul into one dense matmul:
    Wbig[F, G*GO], Wbig[idx[g,f], g*GO+o] += W[g,f,o].  y = x @ Wbig + b.
  - Data-parallel over the batch axis: 8 cores x 32 stocks, 16384 tokens/core.
  - Per core: tile tokens by 128.  PE transposes x tiles ([tok,f] -> [f,tok]),
    then two K=128 float32r matmuls accumulate y[tok, 512] in PSUM.
    ScalarE evicts the transposed tiles PSUM->SBUF; VectorE fuses the bias add
    into the y PSUM->SBUF eviction.  DMAs batched per 1024-token megatile.

Hardcoded shapes: x [256, 512, 256] f32, W [8, 32, 64], b [8, 64], idx [8, 32].
"""

import numpy as np

B, T, F = 256, 512, 256
G, GF, GO = 8, 32, 64
NOUT = G * GO  # 512
N_CORES = 8
NTOK = (B // N_CORES) * T  # 16384 tokens per core
SUB = 128                  # tokens per subtile (partition dim)
LOAD_SUBS = 8              # subtiles per input DMA (1MB, 8KB/partition)
STORE_SUBS = 4             # subtiles per output DMA (1MB, 8KB/partition)
MEGA = SUB * LOAD_SUBS     # 1024 tokens per load block
N_MEGA = NTOK // MEGA      # 16
# Token mapping within a load block: token = tok0 + p*LOAD_SUBS + c
# (partition-major), so each partition's load/store is one contiguous
# HBM chunk (8KB in / 8KB out per partition per DMA).

_CACHE = {}


def _build_module():
    import concourse.mybir as mybir
    import concourse.tile as tile
    from concourse import bacc

    f32 = mybir.dt.float32
    f32r = mybir.dt.float32r

    nc = bacc.Bacc("TRN2", target_bir_lowering=False, debug=False)
    x_d = nc.declare_dram_parameter("x", [NTOK, F], f32, isOutput=False)
    w_d = nc.declare_dram_parameter("w", [128, 2 * NOUT], f32r, isOutput=False)
    b_d = nc.declare_dram_parameter("b", [128, NOUT], f32, isOutput=False)
    id_d = nc.declare_dram_parameter("ident", [128, 128], f32, isOutput=False)
    y_d = nc.declare_dram_parameter("y", [NTOK, NOUT], f32, isOutput=True)

    with tile.TileContext(nc) as tc:
        with (
            tc.tile_pool(name="const", bufs=1) as const_pool,
            tc.tile_pool(name="xin", bufs=6) as xin_pool,
            tc.tile_pool(name="xt", bufs=4) as xt_pool,
            tc.tile_pool(name="yout", bufs=6) as y_pool,
            tc.tile_pool(name="tp", bufs=4, space="PSUM") as tp_pool,
            tc.tile_pool(name="yp", bufs=2, space="PSUM") as yp_pool,
        ):
            w_sb = const_pool.tile([128, 2 * NOUT], f32r)
            nc.gpsimd.dma_start(out=w_sb[:], in_=w_d[:])
            b_sb = const_pool.tile([128, NOUT], f32)
            nc.gpsimd.dma_start(out=b_sb[:], in_=b_d[:])
            id_sb = const_pool.tile([128, 128], f32)
            nc.gpsimd.dma_start(out=id_sb[:], in_=id_d[:])

            for mt in range(N_MEGA):
                tok0 = mt * MEGA
                x_in = xin_pool.tile([128, LOAD_SUBS * F], f32)
                nc.sync.dma_start(
                    out=x_in.rearrange("p (c f) -> p c f", c=LOAD_SUBS),
                    in_=x_d[tok0 : tok0 + MEGA, :].rearrange(
                        "(p c) f -> p c f", p=128
                    ),
                )
                y_sb = None
                for s in range(LOAD_SUBS):
                    if s % STORE_SUBS == 0:
                        y_sb = y_pool.tile([128, STORE_SUBS * NOUT], f32)
                    so = s % STORE_SUBS
                    xt = xt_pool.tile([128, F], f32r)
                    for h in range(2):
                        tp = tp_pool.tile([128, 128], f32)
                        nc.tensor.transpose(
                            tp[:],
                            x_in[:, s * F + h * 128 : s * F + (h + 1) * 128],
                            id_sb[:],
                        )
                        nc.scalar.copy(
                            out=xt[:, h * 128 : (h + 1) * 128], in_=tp[:]
                        )
                    yp = yp_pool.tile([128, NOUT], f32)
                    nc.tensor.matmul(
                        yp[:],
                        lhsT=xt[:, 0:128],
                        rhs=w_sb[:, 0:NOUT],
                        start=True,
                        stop=False,
                    )
                    nc.tensor.matmul(
                        yp[:],
                        lhsT=xt[:, 128:256],
                        rhs=w_sb[:, NOUT : 2 * NOUT],
                        start=False,
                        stop=True,
                    )
                    nc.vector.tensor_add(
                        out=y_sb[:, so * NOUT : (so + 1) * NOUT],
                        in0=yp[:],
                        in1=b_sb[:],
                    )
                    if so == STORE_SUBS - 1:
                        g0 = s - (STORE_SUBS - 1)
                        nc.scalar.dma_start(
                            out=y_d[tok0 : tok0 + MEGA, :].rearrange(
                                "(p c) o -> p c o", p=128
                            )[:, g0 : g0 + STORE_SUBS, :],
                            in_=y_sb.rearrange(
                                "p (c o) -> p c o", c=STORE_SUBS
                            ),
                        )
    nc.finalize()
    return nc


def _get_nc():
    if "nc" not in _CACHE:
        _CACHE["nc"] = _build_module()
    return _CACHE["nc"]


def _prep_inputs(x, W, b, idx):
    x = np.ascontiguousarray(np.asarray(x, dtype=np.float32))
    W = np.asarray(W, dtype=np.float32)
    b = np.asarray(b, dtype=np.float32)
    idx = np.asarray(idx)

    wbig = np.zeros((F, NOUT), dtype=np.float32)
    for g in range(G):
        np.add.at(wbig[:, g * GO : (g + 1) * GO], idx[g].astype(np.int64), W[g])
    w_packed = np.ascontiguousarray(
        np.concatenate([wbig[:128, :], wbig[128:, :]], axis=1)
    )
    b_rep = np.ascontiguousarray(
        np.broadcast_to(b.reshape(1, NOUT), (128, NOUT)).astype(np.float32)
    )
    ident = np.eye(128, dtype=np.float32)

    xs = x.reshape(B * T, F)
    in_maps = []
    for i in range(N_CORES):
        in_maps.append(
            {
                "x": xs[i * NTOK : (i + 1) * NTOK],
                "w": w_packed,
                "b": b_rep,
                "ident": ident,
            }
        )
    return in_maps


def run(inputs, trace=False, **trace_kwargs):
    """Run the SPMD kernel on 8 cores. Returns (full_output, BassKernelResults)."""
    from concourse.bass_utils import run_bass_kernel_spmd

    in_maps = _prep_inputs(
        inputs["x"], inputs["W"], inputs["b"], inputs["idx"]
    )
    nc = _get_nc()
    res = run_bass_kernel_spmd(
        nc, in_maps, list(range(N_CORES)), trace=trace, **trace_kwargs
    )
    out = np.empty((B, T, NOUT), dtype=np.float32)
    bs = B // N_CORES
    for i in range(N_CORES):
        out[i * bs : (i + 1) * bs] = res.results[i]["y"].reshape(bs, T, NOUT)
    return out, res


def kernel(**inputs):
    out, _ = run(inputs, trace=False)
    return out
